# revision 1
# baseline (speedup 1.0000x reference)
"""RGCN (2x hetero GraphConv + mean-pool + MLP) on 8 TRN2 NeuronCores.

Sharding: nodes are dst-sharded 12500/core. Each core owns the aggregation
for its dst rows. Per-edge work is gather (bf16 rows from a replicated
node-feature table in local DRAM) -> scale by the folded degree norm
w_e = rsqrt(deg_in[dst]) * rsqrt(deg_out[src]) -> dma_scatter_add (f32 CCE)
into SBUF accumulators, one relation at a time. Relation weights W_r are
applied per 128-node block (PE transpose + matmul). Layer outputs are
AllGathered so every core has the full table for the next layer's gathers.
Pooling is a per-block matmul against a host-built graph-assignment matrix,
AllReduced across cores, followed by the tiny MLP head (replicated).

The instruction stream is identical on all 8 cores (SPMD); all per-core
variation lives in input tensors (gather/scatter indices, edge weights,
graph assignment). Host-side numpy only computes graph-structure metadata
(degrees/index layouts) and dtype/layout staging of inputs.
"""

import numpy as np
from ml_dtypes import bfloat16

import concourse.bass as bass
import concourse.bacc as bacc
import concourse.mybir as mybir
import concourse.tile as tile
from concourse import bass_utils
from concourse.masks import make_identity

F32 = mybir.dt.float32
BF16 = mybir.dt.bfloat16
I16 = mybir.dt.int16

# problem constants (hardcoded per spec)
N, E, NREL, G, IN, H, C = 100000, 400000, 4, 64, 64, 128, 2
CORES = 8
SHARD = N // CORES            # 12500
NBLK = (SHARD + 127) // 128   # 98
GRP = 25000                   # gather table rows per src-group (int16 idx limit)
NGRP = N // GRP               # 4
CALL = 4096                   # max gather/scatter indices per SWDGE call
# pad scatters land in a dedicated extra accum block at slot NBLK*128


# ---------------------------------------------------------------------------
# host-side planning: pure graph-structure metadata (indices, degrees, layout)
# ---------------------------------------------------------------------------

def _plan(src, dst, graph_ids):
    src = np.asarray(src).astype(np.int64)
    dst = np.asarray(dst).astype(np.int64)
    gid = np.asarray(graph_ids).astype(np.int64)

    # folded normalization: w_e = rsqrt(deg_in[dst]) * rsqrt(deg_out[src])
    w_all = np.empty((NREL, E), np.float32)
    for r in range(NREL):
        do = np.maximum(np.bincount(src[r], minlength=N), 1.0)
        di = np.maximum(np.bincount(dst[r], minlength=N), 1.0)
        w_all[r] = (1.0 / np.sqrt(do[src[r]]) / np.sqrt(di[dst[r]])).astype(np.float32)

    # dma_scatter_add races on duplicate indices within one call, so edges are
    # split into rounds: round k holds the k-th edge of each dst node. One
    # scatter call never spans a round boundary -> indices unique per call.
    rounds_all = {}
    nrounds_max = 0
    for c in range(CORES):
        for r in range(NREL):
            in_core = (dst[r] // SHARD) == c
            for g in range(NGRP):
                sel = np.nonzero(in_core & ((src[r] // GRP) == g))[0]
                order = np.argsort(dst[r][sel], kind="stable")
                sel = sel[order]
                dloc = dst[r][sel] - c * SHARD
                # rank of each edge within its dst group = round index
                if sel.size:
                    first = np.ones(sel.size, np.int64)
                    first[1:] = (np.diff(dloc) != 0).astype(np.int64)
                    run_start = np.nonzero(first)[0]
                    rank = np.arange(sel.size) - np.repeat(
                        run_start, np.diff(np.append(run_start, sel.size)))
                    nr = int(rank.max()) + 1
                    rounds = [sel[rank == k] for k in range(nr)]
                else:
                    rounds = []
                rounds_all[(c, r, g)] = rounds
                nrounds_max = max(nrounds_max, len(rounds))

    # SPMD-uniform round sizes: global max per round index, rounded to 128
    RSZ = []
    for k in range(nrounds_max):
        m = max(len(rounds_all[key][k]) if len(rounds_all[key]) > k else 0
                for key in rounds_all)
        RSZ.append(-(-m // 128) * 128)
    LRG = sum(RSZ)
    ncol = LRG // 16                            # idx columns per run
    nch = LRG // 128                            # chunks per run
    runs = NREL * NGRP
    roff = np.concatenate([[0], np.cumsum(RSZ)]).astype(np.int64)

    gidx = np.zeros((CORES, 16, runs * ncol), np.int16)
    didx = np.full((CORES, 16, runs * ncol), NBLK * 128, np.int16)
    wmeta = np.zeros((CORES, 128, runs * nch), np.float32)

    for c in range(CORES):
        for r in range(NREL):
            for g in range(NGRP):
                run = r * NGRP + g
                rounds = rounds_all[(c, r, g)]
                for k, e in enumerate(rounds):
                    kk = len(e)
                    if kk == 0:
                        continue
                    pos = roff[k] + np.arange(kk)
                    gi = (src[r][e] - (src[r][e] // GRP) * GRP).astype(np.int16)
                    di_ = (dst[r][e] - c * SHARD).astype(np.int16)
                    gidx[c, pos % 16, run * ncol + pos // 16] = gi
                    didx[c, pos % 16, run * ncol + pos // 16] = di_
                    wmeta[c, pos % 128, run * nch + pos // 128] = w_all[r][e]

    # graph assignment matrix with 1/count folded in
    cnt = np.maximum(np.bincount(gid, minlength=G), 1.0)
    gmat = np.zeros((CORES, NBLK * 128, G), np.float32)
    for c in range(CORES):
        ids = gid[c * SHARD:(c + 1) * SHARD]
        gmat[c, np.arange(SHARD), ids] = 1.0 / cnt[ids]

    # gather windows of <= CALL idxs; scatter segments = window cut at round
    # boundaries (so scatter indices are unique within each call)
    windows = []
    off = 0
    while off < LRG:
        s = min(CALL, LRG - off)
        segs = []
        a = off
        while a < off + s:
            b = min(off + s, int(roff[np.searchsorted(roff, a, "right")]))
            segs.append((a - off, b - off))
            a = b
        windows.append((off, s, segs))
        off += s

    # idx tiles span 128 partitions: the 16-row wrap replicated for 8 Q7 cores
    gidx = np.tile(gidx, (1, 8, 1))
    didx = np.tile(didx, (1, 8, 1))
    return dict(LRG=LRG, ncol=ncol, nch=nch, windows=windows,
                gidx=gidx, didx=didx, wmeta=wmeta,
                gmat=gmat.astype(bfloat16))


# ---------------------------------------------------------------------------
# device program
# ---------------------------------------------------------------------------

def _build(plan):
    LRG = plan["LRG"]
    ncol = plan["ncol"]
    nch = plan["nch"]
    windows = plan["windows"]
    runs = NREL * NGRP
    TCH = runs * nch

    nc = bacc.Bacc(None, target_bir_lowering=False, num_devices=CORES)

    # kernel I/O
    p = {}
    p["xT"] = nc.declare_dram_parameter("xT", [IN + 1, N], BF16, isOutput=False)
    p["W65"] = nc.declare_dram_parameter("W65", [IN + 1, H], BF16, isOutput=False)
    p["Wl1"] = nc.declare_dram_parameter("Wl1", [NREL, H, H], BF16, isOutput=False)
    p["Wl2"] = nc.declare_dram_parameter("Wl2", [NREL, H, H], BF16, isOutput=False)
    p["Wm1"] = nc.declare_dram_parameter("Wm1", [H, H], BF16, isOutput=False)
    p["Wm2"] = nc.declare_dram_parameter("Wm2", [H, H], BF16, isOutput=False)
    p["Wm3"] = nc.declare_dram_parameter("Wm3", [H, C], BF16, isOutput=False)
    p["B1"] = nc.declare_dram_parameter("B1", [H, 1], F32, isOutput=False)
    p["B2"] = nc.declare_dram_parameter("B2", [H, 1], F32, isOutput=False)
    p["bm1"] = nc.declare_dram_parameter("bm1", [H, 1], F32, isOutput=False)
    p["bm2"] = nc.declare_dram_parameter("bm2", [H, 1], F32, isOutput=False)
    p["bm3"] = nc.declare_dram_parameter("bm3", [C, 1], F32, isOutput=False)
    p["gidx"] = nc.declare_dram_parameter("gidx", [128, runs * ncol], I16, isOutput=False)
    p["didx"] = nc.declare_dram_parameter("didx", [128, runs * ncol], I16, isOutput=False)
    p["wmeta"] = nc.declare_dram_parameter("wmeta", [128, TCH], F32, isOutput=False)
    p["gmat"] = nc.declare_dram_parameter("gmat", [NBLK * 128, G], BF16, isOutput=False)
    out_ext = nc.declare_dram_parameter("out", [C, G], F32, isOutput=True)

    # internal DRAM
    h0_g = [nc.dram_tensor(f"h0_g{g}", [GRP, H], BF16) for g in range(NGRP)]
    h1_shard = nc.dram_tensor("h1_shard", [SHARD, H], BF16)
    h1_full = nc.dram_tensor("h1_full", [N, H], BF16, addr_space="Shared")
    pool_in = nc.dram_tensor("pool_in", [H, G], F32)
    pool_out = nc.dram_tensor("pool_out", [H, G], F32, addr_space="Shared")

    rg = [list(range(CORES))]

    with tile.TileContext(nc) as tc:
        with (
            tc.tile_pool(name="const", bufs=1) as cpool,
            tc.tile_pool(name="meta", bufs=1) as mpool,
            tc.tile_pool(name="stage", bufs=2) as spool,
            tc.tile_pool(name="scaled", bufs=2) as fpool,
            tc.tile_pool(name="idx", bufs=3) as ipool,
            tc.tile_pool(name="accum", bufs=2) as apool,
            tc.tile_pool(name="out2", bufs=1) as opool,
            tc.tile_pool(name="work", bufs=4) as wpool,
            tc.tile_pool(name="h0s", bufs=2) as hpool,
            tc.tile_pool(name="po", bufs=2, space="PSUM") as po,       # out2 / h0 matmul
            tc.tile_pool(name="pb", bufs=2, space="PSUM") as pb,       # bf16 transpose
            tc.tile_pool(name="pp", bufs=1, space="PSUM") as pp,       # pooled
        ):
            # ---- constants into SBUF
            id_f32 = cpool.tile([128, 128], F32)
            make_identity(nc, id_f32[:])
            id_bf = cpool.tile([128, 128], BF16)
            nc.vector.tensor_copy(id_bf[:], id_f32[:])

            w65 = cpool.tile([IN + 1, H], BF16)
            nc.sync.dma_start(w65[:], p["W65"][:, :])
            wl = {}
            for li, name in ((1, "Wl1"), (2, "Wl2")):
                for r in range(NREL):
                    t = cpool.tile([H, H], BF16, tag=f"wl{li}{r}")
                    nc.sync.dma_start(t[:], p[name][r, :, :])
                    wl[(li, r)] = t
            wm = {}
            for name in ("Wm1", "Wm2"):
                t = cpool.tile([H, H], BF16, tag=name)
                nc.sync.dma_start(t[:], p[name][:, :])
                wm[name] = t
            wm3 = cpool.tile([H, C], BF16)
            nc.sync.dma_start(wm3[:], p["Wm3"][:, :])
            biases = {}
            for name in ("B1", "B2", "bm1", "bm2"):
                t = cpool.tile([H, 1], F32, tag=name)
                nc.sync.dma_start(t[:], p[name][:, :])
                biases[name] = t
            bm3 = cpool.tile([C, 1], F32)
            nc.sync.dma_start(bm3[:], p["bm3"][:, :])

            wmeta = mpool.tile([128, TCH], F32)
            nc.sync.dma_start(wmeta[:], p["wmeta"][:, :])

            # ---- phase 0: h0 = relu(x @ W_in + b_in), node-major, replicated
            STRIP = 4096
            n_strip = -(-N // STRIP)
            ti = 0
            for s in range(n_strip):
                w = min(STRIP, N - s * STRIP)
                strip = hpool.tile([IN + 1, STRIP], BF16, tag="h0strip")
                nc.sync.dma_start(strip[:, :w], p["xT"][:, s * STRIP:s * STRIP + w])
                for q0 in range(0, w, 1024):
                    qw = min(1024, w - q0)
                    nt = -(-qw // 128)
                    hb = wpool.tile([128, 8, H], BF16, tag="h0out")
                    for t in range(nt):
                        t0 = q0 + t * 128
                        tw = min(128, w - t0)
                        ps = po.tile([128, H], F32, tag="mm")
                        nc.tensor.matmul(ps[:tw, :], lhsT=strip[:, t0:t0 + tw],
                                         rhs=w65[:], start=True, stop=True)
                        if ti % 2 == 0:
                            nc.scalar.activation(
                                hb[:tw, t, :], ps[:tw, :],
                                mybir.ActivationFunctionType.Relu)
                        else:
                            nc.vector.tensor_scalar_max(
                                hb[:tw, t, :], ps[:tw, :], 0.0)
                        ti += 1
                    lo = s * STRIP + q0
                    if qw == 1024 and lo // GRP == (lo + qw - 1) // GRP:
                        g0 = lo // GRP
                        a = lo - g0 * GRP
                        nc.sync.dma_start(
                            h0_g[g0][a:a + qw, :].rearrange(
                                "(t p) f -> p t f", p=128),
                            hb[:, :8, :])
                    else:
                        for t in range(nt):
                            t0 = lo + t * 128
                            tw = min(128, s * STRIP + w - t0)
                            done = 0
                            while done < tw:
                                g0 = (t0 + done) // GRP
                                take = min(tw - done,
                                           (g0 + 1) * GRP - (t0 + done))
                                nc.sync.dma_start(
                                    h0_g[g0][t0 + done - g0 * GRP:
                                             t0 + done - g0 * GRP + take, :],
                                    hb[done:done + take, t, :])
                                done += take

            # ---- conv layers
            def conv_layer(layer, tables):
                """layer 1 writes h1_shard (relu+bias+transpose+store);
                layer 2 pools into a PSUM [H, G] accumulator and returns it."""
                # one extra block (slot NBLK*128) soaks up pad scatters.
                # two independent accumulator sets: scatter calls alternate so
                # one set's DMA drain overlaps descriptor-gen for the other.
                accs = [(apool.tile([128, (NBLK + 2) // 2, H], BF16,
                                    name=f"ae{s}", tag=f"ae{s}"),
                         apool.tile([128, (NBLK + 2) // 2, H], BF16,
                                    name=f"ao{s}", tag=f"ao{s}"))
                        for s in range(2)]
                out2 = opool.tile([128, NBLK * H], BF16, tag="out2")
                qn = [0]
                pooled = None
                if layer == 2:
                    pooled = pp.tile([H, G], F32, tag="pooled")
                chunk_id = 0
                for r in range(NREL):
                    # zero accumulators (write-after-read vs prev relation
                    # is ordered by the tile tracker)
                    for s in range(2):
                        nc.vector.memset(accs[s][0][:], 0.0)
                        nc.vector.memset(accs[s][1][:], 0.0)
                    sc_i = 0
                    for g in range(NGRP):
                        run = r * NGRP + g
                        col0 = run * ncol
                        for off, S, segs in windows:
                            ch = S // 128
                            gi = ipool.tile([128, CALL // 16], I16, tag="gi")
                            nc.sync.dma_start(
                                gi[:, :S // 16],
                                p["gidx"][:, col0 + off // 16: col0 + (off + S) // 16])
                            st = spool.tile([128, CALL // 128, H], BF16, tag="st")
                            nc.gpsimd.dma_gather(
                                st[:, :ch, :], tables[g][:, :],
                                gi[:, :S // 16], S, S, H, single_packet=False)
                            sc = fpool.tile([128, CALL // 128, H], BF16, tag="sc")
                            for cc in range(ch):
                                wcol = wmeta[:, chunk_id:chunk_id + 1]
                                if cc % 4 != 3:
                                    nc.scalar.activation(
                                        sc[:, cc, :], st[:, cc, :],
                                        mybir.ActivationFunctionType.Copy,
                                        scale=wcol)
                                else:
                                    nc.vector.tensor_scalar_mul(
                                        sc[:, cc, :], st[:, cc, :], wcol)
                                chunk_id += 1
                            di = ipool.tile([128, CALL // 16], I16, tag="di")
                            nc.sync.dma_start(
                                di[:, :S // 16],
                                p["didx"][:, col0 + off // 16: col0 + (off + S) // 16])
                            for (a, b) in segs:
                                ae, ao = accs[sc_i % 2]
                                sc_i += 1
                                nc.gpsimd.dma_scatter_add(
                                    ae[:, :, :], sc[:, a // 128:b // 128, :],
                                    di[:, a // 16:b // 16], b - a, b - a, H,
                                    sbuf_tokens_per_rank=128, parity_reg=0,
                                    out_ap_other=ao[:, :, :],
                                    single_packet=False)
                    # W_r pass over blocks
                    for b in range(NBLK):
                        pi = b % 2
                        mrg = wpool.tile([128, 128], BF16, tag="mrg")
                        nc.vector.tensor_add(mrg[:], accs[0][pi][:, b // 2, :],
                                             accs[1][pi][:, b // 2, :])
                        tp = pb.tile([128, 128], BF16, tag="wtp")
                        nc.tensor.transpose(tp[:], mrg[:], id_bf[:])
                        aggT = wpool.tile([128, 128], BF16, tag="aggT")
                        nc.scalar.activation(aggT[:], tp[:],
                                             mybir.ActivationFunctionType.Copy)
                        o2 = po.tile([128, 128], F32, tag="mm")
                        nc.tensor.matmul(o2[:], lhsT=wl[(layer, r)][:], rhs=aggT[:],
                                         start=True, stop=True)
                        dsl = out2[:, b * H:(b + 1) * H]
                        if r == 0:
                            nc.vector.tensor_copy(dsl, o2[:])
                        else:
                            nc.vector.tensor_add(dsl, dsl, o2[:])

                # epilogue per block
                for b in range(NBLK):
                    rows = min(128, SHARD - b * 128)
                    sl = out2[:, b * H:(b + 1) * H]
                    if layer == 1:
                        t1 = wpool.tile([128, 128], BF16, tag="t1")
                        nc.scalar.activation(t1[:], sl,
                                             mybir.ActivationFunctionType.Relu,
                                             bias=biases["B1"][:, :])
                        tb = pb.tile([128, 128], BF16, tag="tb")
                        nc.tensor.transpose(tb[:], t1[:], id_bf[:])
                        t2 = wpool.tile([128, 128], BF16, tag="t2")
                        nc.vector.tensor_copy(t2[:], tb[:])
                        nc.sync.dma_start(h1_shard[b * 128:b * 128 + rows, :],
                                          t2[:rows, :])
                    else:
                        tb = pb.tile([128, 128], BF16, tag="tb")
                        nc.tensor.transpose(tb[:], sl, id_bf[:])
                        t2 = wpool.tile([128, 128], BF16, tag="t2")
                        nc.vector.tensor_copy(t2[:], tb[:])
                        gm = wpool.tile([128, G], BF16, tag="gm")
                        nc.sync.dma_start(
                            gm[:rows, :], p["gmat"][b * 128:b * 128 + rows, :])
                        nc.tensor.matmul(pooled[:], lhsT=t2[:rows, :],
                                         rhs=gm[:rows, :],
                                         start=(b == 0), stop=(b == NBLK - 1))
                return pooled

            conv_layer(1, h0_g)
            nc.gpsimd.collective_compute(
                "AllGather", mybir.AluOpType.bypass, replica_groups=rg,
                ins=[h1_shard[:, :]], outs=[h1_full[:, :]])
            pooled = conv_layer(2, [h1_full[g * GRP:(g + 1) * GRP, :] for g in range(NGRP)])

            # ---- pooled partial sums -> AllReduce -> head
            psb = wpool.tile([H, G], F32, tag="psb")
            nc.scalar.activation(psb[:], pooled[:],
                                 mybir.ActivationFunctionType.Copy)
            nc.sync.dma_start(pool_in[:, :], psb[:])
            nc.gpsimd.collective_compute(
                "AllReduce", mybir.AluOpType.add, replica_groups=rg,
                ins=[pool_in[:, :]], outs=[pool_out[:, :]])
            pool_f = wpool.tile([H, G], F32, tag="pool_f")
            nc.sync.dma_start(pool_f[:], pool_out[:, :])
            # + B2 (conv2 bias, linear through the mean), cast to bf16
            pool_b = wpool.tile([H, G], BF16, tag="pool_b")
            nc.vector.tensor_scalar_add(pool_b[:], pool_f[:], biases["B2"][:, :])

            z1p = po.tile([H, G], F32, tag="mm")
            nc.tensor.matmul(z1p[:], lhsT=wm["Wm1"][:], rhs=pool_b[:],
                             start=True, stop=True)
            z1 = wpool.tile([H, G], BF16, tag="z1")
            nc.scalar.activation(z1[:], z1p[:],
                                 mybir.ActivationFunctionType.Relu,
                                 bias=biases["bm1"][:, :])
            z2p = po.tile([H, G], F32, tag="mm")
            nc.tensor.matmul(z2p[:], lhsT=wm["Wm2"][:], rhs=z1[:],
                             start=True, stop=True)
            z2 = wpool.tile([H, G], BF16, tag="z2")
            nc.scalar.activation(z2[:], z2p[:],
                                 mybir.ActivationFunctionType.Relu,
                                 bias=biases["bm2"][:, :])
            z3p = po.tile([C, G], F32, tag="mm")
            nc.tensor.matmul(z3p[:], lhsT=wm3[:], rhs=z2[:],
                             start=True, stop=True)
            z3 = wpool.tile([C, G], F32, tag="z3")
            nc.vector.tensor_scalar_add(z3[:], z3p[:], bm3[:, :])
            nc.sync.dma_start(out_ext[:, :], z3[:])

    nc.compile()
    return nc


# ---------------------------------------------------------------------------
# entry point
# ---------------------------------------------------------------------------

_CACHE = {}


def kernel(x, src, dst, graph_ids, W_in, b_in, W1, b1, W2, b2,
           Wm1, bm1, Wm2, bm2, Wm3, bm3):
    x = np.asarray(x)
    key = (int(np.asarray(src).sum()) ^ int(np.asarray(dst).sum()),
           int(np.asarray(graph_ids).sum()))
    if key not in _CACHE:
        plan = _plan(src, dst, graph_ids)
        nc = _build(plan)
        _CACHE[key] = (plan, nc)
    plan, nc = _CACHE[key]

    xT = np.concatenate([np.asarray(x).T, np.ones((1, N), np.float32)], axis=0)
    w65 = np.concatenate([np.asarray(W_in), np.asarray(b_in)[None, :]], axis=0)

    def bf(a):
        return np.ascontiguousarray(np.asarray(a), dtype=np.float32).astype(bfloat16)

    def col(a):
        return np.ascontiguousarray(np.asarray(a, np.float32).reshape(-1, 1))

    in_maps = []
    for c in range(CORES):
        in_maps.append({
            "xT": bf(xT),
            "W65": bf(w65),
            "Wl1": bf(W1),
            "Wl2": bf(W2),
            "Wm1": bf(Wm1),
            "Wm2": bf(Wm2),
            "Wm3": bf(Wm3),
            "B1": col(np.asarray(b1, np.float32).sum(axis=0)),
            "B2": col(np.asarray(b2, np.float32).sum(axis=0)),
            "bm1": col(bm1),
            "bm2": col(bm2),
            "bm3": col(bm3),
            "gidx": np.ascontiguousarray(plan["gidx"][c]),
            "didx": np.ascontiguousarray(plan["didx"][c]),
            "wmeta": np.ascontiguousarray(plan["wmeta"][c]),
            "gmat": np.ascontiguousarray(plan["gmat"][c]),
        })

    res = bass_utils.run_bass_kernel_spmd(nc, in_maps, list(range(CORES)))
    global LAST_EXEC_NS
    LAST_EXEC_NS = res.exec_time_ns
    out = np.asarray(res.results[0]["out"], np.float32)  # [C, G]
    return np.ascontiguousarray(out.T)                   # [G, C]


LAST_EXEC_NS = None


if __name__ == "__main__":
    import reference
    import jax
    with jax.default_device(jax.devices("cpu")[0]):
        inp = {k: np.asarray(v) for k, v in reference.setup_inputs().items()}
        exp = np.asarray(reference.reference(**{k: v for k, v in inp.items()}))
    act = kernel(**inp)
    rel = np.linalg.norm(act - exp) / np.linalg.norm(exp)
    print("Relative error:", rel)



# revision 6
# speedup vs baseline: 1.7206x; 1.7206x over previous
"""RGCN (2x hetero GraphConv + mean-pool + MLP) on 8 TRN2 NeuronCores — v2.

Nodes are dst-sharded 12500/core. Per layer, each core aggregates its dst
rows: edges are sorted by (dst-block, src-group) with all 4 relations
merged; source features are fetched per-edge with SWDGE dma_gather (int16
indices, 4 src groups of 25000 rows) and reduced into per-dst-block PSUM
accumulators [H, 4*128] (f32) by one-hot PE matmuls — column r*128+dslot
carries edge weight selection, so no scatter DMA exists at all. Relation
weights W_r then contract the accumulator slices; layer 1 applies
relu+bias and stores its shard (AllGather -> full table for layer 2);
layer 2 feeds mean-pooling (matmul with a host-built graph-assignment
matrix), AllReduce, and the small MLP head.

The instruction stream is identical on all 8 cores (SPMD); per-core
variation lives in input tensors (indices, one-hot metadata, graph
assignment). Host-side numpy only computes graph-structure metadata
(degrees/index layouts) and dtype/layout staging of inputs.
"""

import numpy as np
from ml_dtypes import bfloat16

import concourse.bass as bass
import concourse.bacc as bacc
import concourse.mybir as mybir
import concourse.tile as tile
from concourse import bass_utils
from concourse.masks import make_identity

F32 = mybir.dt.float32
F16 = mybir.dt.float16
BF16 = mybir.dt.bfloat16
I16 = mybir.dt.int16
I32 = mybir.dt.int32

# problem constants (hardcoded per spec)
N, E, NREL, G, IN, H, C = 100000, 400000, 4, 64, 64, 128, 2
CORES = 8
SHARD = N // CORES            # 12500
NBLK = (SHARD + 127) // 128   # 98
GRP = 25000                   # src rows per gather group (int16 idx limit)
NGRP = N // GRP               # 4
OW = NREL * 128               # one-hot width (relation-merged): 512
SBS = 4                       # dst blocks per superblock (PSUM banks)
KMAX = 32                     # max 128-edge chunks per gather call


# ---------------------------------------------------------------------------
# host-side planning: pure graph-structure metadata (indices, degrees, layout)
# ---------------------------------------------------------------------------

def _plan(src, dst, graph_ids):
    src = np.asarray(src).astype(np.int64)
    dst = np.asarray(dst).astype(np.int64)
    gid = np.asarray(graph_ids).astype(np.int64)

    # folded normalization: w_e = rsqrt(deg_in[dst]) * rsqrt(deg_out[src])
    w_all = np.empty((NREL, E), np.float32)
    for r in range(NREL):
        do = np.maximum(np.bincount(src[r], minlength=N), 1.0)
        di = np.maximum(np.bincount(dst[r], minlength=N), 1.0)
        w_all[r] = (1.0 / np.sqrt(do[src[r]]) / np.sqrt(di[dst[r]])).astype(np.float32)

    # per (core, block, group) cells; relations merged inside a cell
    cells = {}
    cnt = np.zeros((CORES, NBLK, NGRP), np.int64)
    for c in range(CORES):
        for b in range(NBLK):
            for g in range(NGRP):
                cells[(c, b, g)] = []
        for r in range(NREL):
            local = dst[r] - c * SHARD
            m = (local >= 0) & (local < SHARD)
            es = np.nonzero(m)[0]
            loc = local[es]
            b_arr = loc // 128
            g_arr = src[r][es] // GRP
            key = b_arr * NGRP + g_arr
            order = np.argsort(key, kind="stable")
            es, loc, b_arr, g_arr = es[order], loc[order], b_arr[order], g_arr[order]
            keys = b_arr * NGRP + g_arr
            bounds = np.searchsorted(keys, np.arange(NBLK * NGRP + 1))
            for cell in range(NBLK * NGRP):
                s0, s1 = bounds[cell], bounds[cell + 1]
                if s0 == s1:
                    continue
                b, g = divmod(cell, NGRP)
                ee = es[s0:s1]
                cells[(c, b, g)].append((
                    (src[r][ee] - g * GRP).astype(np.int16),
                    (r * 128 + loc[s0:s1] - b * 128).astype(np.float16),
                    w_all[r][ee].astype(np.float32),
                ))
                cnt[c, b, g] += s1 - s0

    # SPMD-uniform chunk counts: global max per cell, in 128-edge chunks
    nch = -(-cnt.max(axis=0) // 128)            # [NBLK, NGRP]

    # emission schedule: superblocks of SBS dst blocks; per sb, per group,
    # chunks of that (b in sb, g) run; gather calls batch <= KMAX chunks
    sbs = []
    q = 0
    col = 0
    for s0 in range(0, NBLK, SBS):
        blocks = list(range(s0, min(s0 + SBS, NBLK)))
        tb = {b: int(nch[b, :].sum()) for b in blocks}
        seen = {b: 0 for b in blocks}
        calls = []
        for g in range(NGRP):
            chunk_list = []
            for b in blocks:
                for j in range(int(nch[b, g])):
                    chunk_list.append(b)
            for i0 in range(0, len(chunk_list), KMAX):
                grp_chunks = chunk_list[i0:i0 + KMAX]
                kc = len(grp_chunks)
                info = []
                for b in grp_chunks:
                    info.append((b, seen[b] == 0, seen[b] == tb[b] - 1))
                    seen[b] += 1
                calls.append(dict(g=g, kc=kc, q0=q, col0=col, chunks=info))
                q += kc
                col += kc * 8
        sbs.append(dict(blocks=blocks, calls=calls, tb=tb))
    NCH = q
    NIDXCOL = col

    IDX = np.zeros((CORES, 16, NIDXCOL), np.int16)
    DSL = np.full((CORES, 128, NCH), -1.0, np.float16)
    WC = np.zeros((CORES, 128, NCH), np.float32)
    for c in range(CORES):
        # per-cell chunked fill, following the same emission schedule
        cursors = {}
        for sb in sbs:
            for call in sb["calls"]:
                g = call["g"]
                for i, (b, _, _) in enumerate(call["chunks"]):
                    qq = call["q0"] + i
                    key = (c, b, g)
                    if key not in cursors:
                        parts = cells[key]
                        if parts:
                            cursors[key] = [
                                np.concatenate([p[0] for p in parts]),
                                np.concatenate([p[1] for p in parts]),
                                np.concatenate([p[2] for p in parts]),
                                0]
                        else:
                            cursors[key] = [np.zeros(0, np.int16),
                                            np.zeros(0, np.float16),
                                            np.zeros(0, np.float32), 0]
                    si, dl, wv, pos = cursors[key]
                    take = min(128, len(si) - pos)
                    if take > 0:
                        sl = slice(pos, pos + take)
                        # gathered row n -> out[n%128, n//128]; n = i*128 + p
                        nn = i * 128 + np.arange(take)
                        IDX[c, nn % 16, call["col0"] + nn // 16] = si[sl]
                        DSL[c, :take, qq] = dl[sl]
                        WC[c, :take, qq] = wv[sl]
                        cursors[key][3] = pos + take

    # graph assignment matrix with 1/count folded in
    cnt_g = np.maximum(np.bincount(gid, minlength=G), 1.0)
    gmat = np.zeros((CORES, NBLK * 128, G), np.float32)
    for c in range(CORES):
        ids = gid[c * SHARD:(c + 1) * SHARD]
        gmat[c, np.arange(SHARD), ids] = 1.0 / cnt_g[ids]

    return dict(sbs=sbs, NCH=NCH, NIDXCOL=NIDXCOL,
                IDX=np.tile(IDX, (1, 8, 1)),
                DSL=DSL, WC=WC.astype(bfloat16),
                gmat=gmat.astype(bfloat16))


# ---------------------------------------------------------------------------
# device program
# ---------------------------------------------------------------------------

def _build(plan):
    sbs = plan["sbs"]
    NCH = plan["NCH"]
    NIDXCOL = plan["NIDXCOL"]

    nc = bacc.Bacc(None, target_bir_lowering=False, num_devices=CORES)

    p = {}
    p["xTs"] = nc.declare_dram_parameter("xTs", [IN + 1, SHARD], BF16, isOutput=False)
    p["W65"] = nc.declare_dram_parameter("W65", [IN + 1, H], BF16, isOutput=False)
    p["Wl1"] = nc.declare_dram_parameter("Wl1", [NREL, H, H], BF16, isOutput=False)
    p["Wl2"] = nc.declare_dram_parameter("Wl2", [NREL, H, H], BF16, isOutput=False)
    p["Wm1"] = nc.declare_dram_parameter("Wm1", [H, H], BF16, isOutput=False)
    p["Wm2"] = nc.declare_dram_parameter("Wm2", [H, H], BF16, isOutput=False)
    p["Wm3"] = nc.declare_dram_parameter("Wm3", [H, C], BF16, isOutput=False)
    p["B1"] = nc.declare_dram_parameter("B1", [H, 1], F32, isOutput=False)
    p["B2"] = nc.declare_dram_parameter("B2", [H, 1], F32, isOutput=False)
    p["bm1"] = nc.declare_dram_parameter("bm1", [H, 1], F32, isOutput=False)
    p["bm2"] = nc.declare_dram_parameter("bm2", [H, 1], F32, isOutput=False)
    p["bm3"] = nc.declare_dram_parameter("bm3", [C, 1], F32, isOutput=False)
    p["IDX"] = nc.declare_dram_parameter("IDX", [128, NIDXCOL], I16, isOutput=False)
    p["DSL"] = nc.declare_dram_parameter("DSL", [128, NCH], F16, isOutput=False)
    p["WC"] = nc.declare_dram_parameter("WC", [128, NCH], BF16, isOutput=False)
    p["gmat"] = nc.declare_dram_parameter("gmat", [NBLK * 128, G], BF16, isOutput=False)
    out_ext = nc.declare_dram_parameter("out", [C, G], F32, isOutput=True)

    h0_shard = nc.dram_tensor("h0_shard", [SHARD, H], BF16)
    h0_full = nc.dram_tensor("h0_full", [N, H], BF16, addr_space="Shared")
    h1_shard = nc.dram_tensor("h1_shard", [SHARD, H], BF16)
    h1_full = nc.dram_tensor("h1_full", [N, H], BF16, addr_space="Shared")
    pool_in = nc.dram_tensor("pool_in", [H, G], F32)
    pool_out = nc.dram_tensor("pool_out", [H, G], F32, addr_space="Shared")

    rg = [list(range(CORES))]

    with tile.TileContext(nc) as tc:
        with (
            tc.tile_pool(name="const", bufs=1) as cpool,
            tc.tile_pool(name="idx", bufs=3) as ipool,
            tc.tile_pool(name="meta", bufs=3) as mpool,
            tc.tile_pool(name="gath", bufs=3) as dpool,
            tc.tile_pool(name="scal", bufs=2) as spool,
            tc.tile_pool(name="oneh", bufs=2) as opool,
            tc.tile_pool(name="aggs", bufs=2) as apool,
            tc.tile_pool(name="work", bufs=4) as wpool,
            tc.tile_pool(name="pa", bufs=1, space="PSUM") as pa,
            tc.tile_pool(name="po", bufs=2, space="PSUM") as po,
            tc.tile_pool(name="pb", bufs=1, space="PSUM") as pb,
            tc.tile_pool(name="pp", bufs=1, space="PSUM") as pp,
        ):
            # ---- constants
            id_f32 = cpool.tile([128, 128], F32)
            make_identity(nc, id_f32[:])
            id_bf = cpool.tile([128, 128], BF16)
            nc.vector.tensor_copy(id_bf[:], id_f32[:])

            iota_i = cpool.tile([128, OW], I32)
            nc.gpsimd.iota(iota_i[:], pattern=[[1, OW]], base=0,
                           channel_multiplier=0)
            iota_f = cpool.tile([128, OW], F16)
            nc.vector.tensor_copy(iota_f[:], iota_i[:])

            w65 = cpool.tile([IN + 1, H], BF16)
            nc.sync.dma_start(w65[:], p["W65"][:, :])
            wl = {}
            for li, name in ((1, "Wl1"), (2, "Wl2")):
                for r in range(NREL):
                    t = cpool.tile([H, H], BF16, tag=f"wl{li}{r}")
                    nc.sync.dma_start(t[:], p[name][r, :, :])
                    wl[(li, r)] = t
            wm = {}
            for name in ("Wm1", "Wm2"):
                t = cpool.tile([H, H], BF16, tag=name)
                nc.sync.dma_start(t[:], p[name][:, :])
                wm[name] = t
            wm3 = cpool.tile([H, C], BF16)
            nc.sync.dma_start(wm3[:], p["Wm3"][:, :])
            biases = {}
            for name in ("B1", "B2", "bm1", "bm2"):
                t = cpool.tile([H, 1], F32, tag=name)
                nc.sync.dma_start(t[:], p[name][:, :])
                biases[name] = t
            bm3 = cpool.tile([C, 1], F32)
            nc.sync.dma_start(bm3[:], p["bm3"][:, :])

            # ---- phase 0: h0 = relu(x @ W_in + b_in) for this core's shard
            ti = 0
            with tc.tile_pool(name="ph0", bufs=1) as hpool:
                xs = hpool.tile([IN + 1, SHARD], BF16)
                nc.sync.dma_start(xs[:], p["xTs"][:, :])
                for t in range(NBLK):
                    t0 = t * 128
                    tw = min(128, SHARD - t0)
                    ps = po.tile([128, H], F32, tag="mm")
                    nc.tensor.matmul(ps[:tw, :], lhsT=xs[:, t0:t0 + tw],
                                     rhs=w65[:], start=True, stop=True)
                    hb = wpool.tile([128, H], BF16, tag="h0out")
                    if ti % 2 == 0:
                        nc.scalar.activation(hb[:tw, :], ps[:tw, :],
                                             mybir.ActivationFunctionType.Relu)
                    else:
                        nc.vector.tensor_scalar_max(hb[:tw, :], ps[:tw, :], 0.0)
                    ti += 1
                    nc.sync.dma_start(h0_shard[t0:t0 + tw, :], hb[:tw, :])

            nc.gpsimd.collective_compute(
                "AllGather", mybir.AluOpType.bypass, replica_groups=rg,
                ins=[h0_shard[:, :]], outs=[h0_full[:, :]])

            # ---- conv layers
            def conv(layer, h_full):
                tables = [h_full[g * GRP:(g + 1) * GRP, :] for g in range(NGRP)]
                pooled = None
                if layer == 2:
                    pooled = pp.tile([H, G], F32, tag="pooled")
                ei = 0
                for sb in sbs:
                    agg = {}
                    for call in sb["calls"]:
                        g, kc, q0, col0 = (call["g"], call["kc"],
                                           call["q0"], call["col0"])
                        it = ipool.tile([128, KMAX * 8], I16, tag="it")
                        nc.sync.dma_start(it[:, :kc * 8],
                                          p["IDX"][:, col0:col0 + kc * 8])
                        st = dpool.tile([128, KMAX, H], BF16, tag="st")
                        nc.gpsimd.dma_gather(
                            st[:, :kc, :], tables[g], it[:, :kc * 8],
                            kc * 128, kc * 128, H, single_packet=False)
                        dl = mpool.tile([128, KMAX], F16, tag="dl")
                        nc.scalar.dma_start(dl[:, :kc], p["DSL"][:, q0:q0 + kc])
                        wc = mpool.tile([128, KMAX], BF16, tag="wc")
                        nc.scalar.dma_start(wc[:, :kc], p["WC"][:, q0:q0 + kc])
                        oh = opool.tile([128, KMAX, OW], BF16, tag="oh")
                        nc.vector.tensor_tensor(
                            out=oh[:, :kc, :],
                            in0=dl[:, :kc].to_broadcast([128, kc, OW]),
                            in1=iota_f[:, :].rearrange(
                                "p (o f) -> p o f", o=1).broadcast_to(
                                    [128, kc, OW]),
                            op=mybir.AluOpType.is_equal)
                        sc = spool.tile([128, KMAX, H], BF16, tag="sc")
                        nc.vector.tensor_tensor(
                            out=sc[:, :kc, :], in0=st[:, :kc, :],
                            in1=wc[:, :kc].to_broadcast([128, kc, H]),
                            op=mybir.AluOpType.mult)
                        for i, (b, first, last) in enumerate(call["chunks"]):
                            if b not in agg:
                                agg[b] = pa.tile([128, OW], F32,
                                                 name=f"agg{b % SBS}",
                                                 tag=f"agg{b % SBS}")
                            nc.tensor.matmul(agg[b][:], lhsT=sc[:, i, :],
                                             rhs=oh[:, i, :],
                                             start=first, stop=last)
                    # superblock epilogues
                    for b in sb["blocks"]:
                        rows = min(128, SHARD - b * 128)
                        ag4 = apool.tile([128, OW], BF16, tag=f"as{b % 2}")
                        if sb["tb"][b] == 0:
                            nc.vector.memset(ag4[:], 0.0)
                        elif ei % 2 == 0:
                            nc.scalar.activation(
                                ag4[:], agg[b][:],
                                mybir.ActivationFunctionType.Copy)
                        else:
                            nc.vector.tensor_copy(ag4[:], agg[b][:])
                        ei += 1
                        out2 = po.tile([128, 128], F32, tag="mm")
                        for r in range(NREL):
                            nc.tensor.matmul(
                                out2[:], lhsT=wl[(layer, r)][:],
                                rhs=ag4[:, r * 128:(r + 1) * 128],
                                start=(r == 0), stop=(r == NREL - 1))
                        if layer == 1:
                            t1 = wpool.tile([128, 128], BF16, tag="t1")
                            nc.scalar.activation(
                                t1[:], out2[:],
                                mybir.ActivationFunctionType.Relu,
                                bias=biases["B1"][:, :])
                            tb_ = pb.tile([128, 128], BF16, tag="tb")
                            nc.tensor.transpose(tb_[:], t1[:], id_bf[:])
                            t2 = wpool.tile([128, 128], BF16, tag="t2")
                            nc.vector.tensor_copy(t2[:], tb_[:])
                            nc.sync.dma_start(
                                h1_shard[b * 128:b * 128 + rows, :],
                                t2[:rows, :])
                        else:
                            t1 = wpool.tile([128, 128], BF16, tag="t1")
                            nc.scalar.activation(
                                t1[:], out2[:],
                                mybir.ActivationFunctionType.Copy)
                            tb_ = pb.tile([128, 128], BF16, tag="tb")
                            nc.tensor.transpose(tb_[:], t1[:], id_bf[:])
                            t2 = wpool.tile([128, 128], BF16, tag="t2")
                            nc.vector.tensor_copy(t2[:], tb_[:])
                            gm = wpool.tile([128, G], BF16, tag="gm")
                            nc.scalar.dma_start(
                                gm[:rows, :],
                                p["gmat"][b * 128:b * 128 + rows, :])
                            nc.tensor.matmul(pooled[:], lhsT=t2[:rows, :],
                                             rhs=gm[:rows, :],
                                             start=(b == 0),
                                             stop=(b == NBLK - 1))
                return pooled

            conv(1, h0_full)
            nc.gpsimd.collective_compute(
                "AllGather", mybir.AluOpType.bypass, replica_groups=rg,
                ins=[h1_shard[:, :]], outs=[h1_full[:, :]])
            pooled = conv(2, h1_full)

            # ---- pooled partial sums -> AllReduce -> head
            psb = wpool.tile([H, G], F32, tag="psb")
            nc.scalar.activation(psb[:], pooled[:],
                                 mybir.ActivationFunctionType.Copy)
            nc.sync.dma_start(pool_in[:, :], psb[:])
            nc.gpsimd.collective_compute(
                "AllReduce", mybir.AluOpType.add, replica_groups=rg,
                ins=[pool_in[:, :]], outs=[pool_out[:, :]])
            pool_f = wpool.tile([H, G], F32, tag="pool_f")
            nc.sync.dma_start(pool_f[:], pool_out[:, :])
            # + B2 (conv2 bias, linear through the mean), cast to bf16
            pool_b = wpool.tile([H, G], BF16, tag="pool_b")
            nc.vector.tensor_scalar_add(pool_b[:], pool_f[:], biases["B2"][:, :])

            z1p = po.tile([H, G], F32, tag="mm")
            nc.tensor.matmul(z1p[:], lhsT=wm["Wm1"][:], rhs=pool_b[:],
                             start=True, stop=True)
            z1 = wpool.tile([H, G], BF16, tag="z1")
            nc.scalar.activation(z1[:], z1p[:],
                                 mybir.ActivationFunctionType.Relu,
                                 bias=biases["bm1"][:, :])
            z2p = po.tile([H, G], F32, tag="mm")
            nc.tensor.matmul(z2p[:], lhsT=wm["Wm2"][:], rhs=z1[:],
                             start=True, stop=True)
            z2 = wpool.tile([H, G], BF16, tag="z2")
            nc.scalar.activation(z2[:], z2p[:],
                                 mybir.ActivationFunctionType.Relu,
                                 bias=biases["bm2"][:, :])
            z3p = po.tile([C, G], F32, tag="mm")
            nc.tensor.matmul(z3p[:], lhsT=wm3[:], rhs=z2[:],
                             start=True, stop=True)
            z3 = wpool.tile([C, G], F32, tag="z3")
            nc.vector.tensor_scalar_add(z3[:], z3p[:], bm3[:, :])
            nc.sync.dma_start(out_ext[:, :], z3[:])

    nc.compile()
    return nc


# ---------------------------------------------------------------------------
# entry point
# ---------------------------------------------------------------------------

_CACHE = {}


def kernel(x, src, dst, graph_ids, W_in, b_in, W1, b1, W2, b2,
           Wm1, bm1, Wm2, bm2, Wm3, bm3):
    x = np.asarray(x)
    key = (int(np.asarray(src).sum()) ^ int(np.asarray(dst).sum()),
           int(np.asarray(graph_ids).sum()))
    if key not in _CACHE:
        plan = _plan(src, dst, graph_ids)
        nc = _build(plan)
        _CACHE[key] = (plan, nc)
    plan, nc = _CACHE[key]

    xT = np.concatenate([np.asarray(x).T, np.ones((1, N), np.float32)], axis=0)
    w65 = np.concatenate([np.asarray(W_in), np.asarray(b_in)[None, :]], axis=0)

    def bf(a):
        return np.ascontiguousarray(np.asarray(a), dtype=np.float32).astype(bfloat16)

    def col(a):
        return np.ascontiguousarray(np.asarray(a, np.float32).reshape(-1, 1))

    xTb = bf(xT)
    in_maps = []
    for c in range(CORES):
        in_maps.append({
            "xTs": np.ascontiguousarray(xTb[:, c * SHARD:(c + 1) * SHARD]),
            "W65": bf(w65),
            "Wl1": bf(W1),
            "Wl2": bf(W2),
            "Wm1": bf(Wm1),
            "Wm2": bf(Wm2),
            "Wm3": bf(Wm3),
            "B1": col(np.asarray(b1, np.float32).sum(axis=0)),
            "B2": col(np.asarray(b2, np.float32).sum(axis=0)),
            "bm1": col(bm1),
            "bm2": col(bm2),
            "bm3": col(bm3),
            "IDX": np.ascontiguousarray(plan["IDX"][c]),
            "DSL": np.ascontiguousarray(plan["DSL"][c]),
            "WC": np.ascontiguousarray(plan["WC"][c]),
            "gmat": np.ascontiguousarray(plan["gmat"][c]),
        })

    res = bass_utils.run_bass_kernel_spmd(nc, in_maps, list(range(CORES)))
    global LAST_EXEC_NS
    LAST_EXEC_NS = res.exec_time_ns
    out = np.asarray(res.results[0]["out"], np.float32)  # [C, G]
    return np.ascontiguousarray(out.T)                   # [G, C]


LAST_EXEC_NS = None


if __name__ == "__main__":
    import reference
    import jax
    with jax.default_device(jax.devices("cpu")[0]):
        inp = {k: np.asarray(v) for k, v in reference.setup_inputs().items()}
        exp = np.asarray(reference.reference(**{k: v for k, v in inp.items()}))
    act = kernel(**inp)
    rel = np.linalg.norm(act - exp) / np.linalg.norm(exp)
    print("Relative error:", rel)
